# revision 2
# baseline (speedup 1.0000x reference)
"""Causal self-attention (B=2, T=2048, C=1024, H=16) on 8 trn2 NeuronCores.

V6: head-pair tensor parallelism + one 8-way AllToAll.

Core c owns head pair c (heads 2c, 2c+1; qkv channels 128c..128(c+1))
and computes QKV + full causal attention for BOTH batches (4.3 GFLOP
vs 13.4 GFLOP/core for the collective-free row-sharded V5). The pair's
attention output aot[b] = [128 dims, 2048 q] bf16 is exchanged with one
8-way AllToAll (shard j = aot[j//4][:, 512*(j%4):...] -> core j), after
which core c holds all 16 heads for its own 512 output rows
(batch c//4, rows 512*(c%4)+0..511) and runs the projection locally.

Attention runs per (batch, q-chunk qc of 512) over 2-k-tile groups
(2qc+2 groups): per group, 4 score matmuls (halves interleaved so the
two heads' 64-contract matmuls occupy disjoint PE row groups), causal
masking IN PSUM via matmul-adds of -30000 constants (tri for diagonal
128x128 blocks, full-block fill for the dead block of the odd k-tile),
one exp activation per half covering both k-tiles ([128, 2, N] AP,
PSUM->SBUF bf16), AV with a ones column in V accumulating the softmax
denominator into pav row 64. pav is released by a single [65,1024]
staging copy; the reciprocal/broadcast/normalize chain runs deferred,
off the AV critical path. V/K/Q of batch 1 are issued inside batch 0's
attention stream to fill PE gaps.
"""

import sys

for _p in ("/opt/trn_rl_repo", "/root/.axon_site/_ro/trn_rl_repo"):
    if _p not in sys.path:
        sys.path.append(_p)

import numpy as np

import concourse.bass as bass
import concourse.mybir as mybir
import concourse.tile as tile
from concourse import bacc

DIM = 1024
N_HEADS = 16
B = 2
T = 2048
KT = DIM // 128   # 8 contract chunks
TT = T // 128     # 16 position tiles
QC = T // 512     # 4 q-chunks
SCALE = 1.0 / 8.0
N_CORES = 8

F32R = mybir.dt.float32r
BF16 = mybir.dt.bfloat16
F32 = mybir.dt.float32

_CACHE = {}


def build_nc(phases="all", reps=1):
    nc = bacc.Bacc(None)

    xT = [nc.dram_tensor(f"xT{b}", [128, KT * T], BF16, kind="ExternalInput")
          for b in range(B)]
    wq = nc.dram_tensor("wq", [128, KT * 128], BF16, kind="ExternalInput")
    wk = nc.dram_tensor("wk", [128, KT * 128], BF16, kind="ExternalInput")
    wv = nc.dram_tensor("wv", [128, KT * 128], BF16, kind="ExternalInput")
    wo = nc.dram_tensor("wo", [128, KT * DIM], BF16, kind="ExternalInput")
    tri = nc.dram_tensor("tri", [128, 128], BF16, kind="ExternalInput")
    allm = nc.dram_tensor("allm", [128, 128], BF16, kind="ExternalInput")
    ident = nc.dram_tensor("ident", [128, 128], BF16, kind="ExternalInput")
    ones_bf = nc.dram_tensor("ones_bf", [128, 64], BF16, kind="ExternalInput")
    ones_fr = nc.dram_tensor("ones_fr", [128, 64], F32R, kind="ExternalInput")
    out = nc.dram_tensor("out", [512, DIM], F32, kind="ExternalOutput")

    do_attn = phases in ("attn", "attnx", "attne", "all")
    dummy_e = phases == "attnx"   # skip exp; AV reads a fixed tile
    free_av = phases in ("attnx", "attne")  # AV does not wait on exp
    do_proj = phases == "all"

    with tile.TileContext(nc) as tc:
        with tc.tile_pool(name="pers", bufs=1) as pers, \
             tc.tile_pool(name="work", bufs=3) as work, \
             tc.tile_pool(name="epool", bufs=4) as epool, \
             tc.tile_pool(name="pst", bufs=1, space="PSUM") as pst, \
             tc.tile_pool(name="pmm", bufs=2, space="PSUM") as pmm, \
             tc.tile_pool(name="pav", bufs=1, space="PSUM") as pav_pool:

            x_sb = [pers.tile([128, KT * T], BF16, tag=f"x{b}", name=f"x{b}")
                    for b in range(B)]
            x4 = [x_sb[b][:].rearrange("p (t k i) -> p t k i", t=TT, k=KT)
                  for b in range(B)]
            ktp = [pers.tile([128, T], BF16, tag=f"ktp{b}", name=f"ktp{b}")
                   for b in range(B)]
            qtp = [pers.tile([128, T], BF16, tag=f"qtp{b}", name=f"qtp{b}")
                   for b in range(B)]
            v_sb = [pers.tile([128, TT * 130], BF16, tag=f"v{b}", name=f"v{b}")
                    for b in range(B)]
            v4 = [v_sb[b][:].rearrange("p (t h e) -> p t h e", t=TT, h=2, e=65)
                  for b in range(B)]
            aot = [pers.tile([128, T], BF16, tag=f"aot{b}", name=f"aot{b}")
                   for b in range(B)]
            wq_sb = pers.tile([128, KT * 128], BF16, tag="wq")
            wk_sb = pers.tile([128, KT * 128], BF16, tag="wk")
            wv_sb = pers.tile([128, KT * 128], BF16, tag="wv")
            wo_sb = pers.tile([128, KT * DIM], BF16, tag="wo")
            tri_sb = pers.tile([128, 128], BF16, tag="tri")
            allm_sb = pers.tile([128, 128], BF16, tag="allm")
            id_sb = pers.tile([128, 128], BF16, tag="ident")
            onesb_sb = pers.tile([128, 64], BF16, tag="onesb")
            onesf_sb = pers.tile([128, 64], F32R, tag="onesf")
            pin = [pers.tile([128, 512], BF16, tag=f"pin{s}", name=f"pin{s}")
                   for s in range(8)]
            efix = pers.tile([128, 1024], BF16, tag="efix")

            for _rep in range(reps):
                # ---- DMA preamble ----
                nc.sync.dma_start(out=wv_sb[:], in_=wv[:])
                for tt in range(TT):  # x0 tt-major: V(b0) streams behind it
                    nc.sync.dma_start(
                        out=x_sb[0][:, tt * 1024:(tt + 1) * 1024],
                        in_=xT[0][:, tt * 1024:(tt + 1) * 1024])
                nc.sync.dma_start(out=wk_sb[:], in_=wk[:])
                nc.sync.dma_start(out=wq_sb[:], in_=wq[:])
                nc.sync.dma_start(out=tri_sb[:], in_=tri[:])
                nc.sync.dma_start(out=allm_sb[:], in_=allm[:])
                nc.sync.dma_start(out=id_sb[:], in_=ident[:])
                nc.sync.dma_start(out=onesb_sb[:], in_=ones_bf[:])
                nc.sync.dma_start(out=onesf_sb[:], in_=ones_fr[:])
                for tt in range(TT):
                    nc.sync.dma_start(
                        out=x_sb[1][:, tt * 1024:(tt + 1) * 1024],
                        in_=xT[1][:, tt * 1024:(tt + 1) * 1024])
                if do_proj:
                    for q4 in range(4):
                        nc.sync.dma_start(
                            out=wo_sb[:, q4 * 2048:(q4 + 1) * 2048],
                            in_=wo[:, q4 * 2048:(q4 + 1) * 2048])

                def v_pass(b, tt0, tt1):
                    for tt in range(tt0, tt1):
                        vacc = pmm.tile([128, 512], F32, tag="mm",
                                        name=f"vacc_{b}_{tt}_{_rep}")
                        for kt in range(KT):
                            nc.tensor.matmul(
                                vacc[:, 0:128],
                                x4[b][:, tt, kt, :],
                                wv_sb[:, kt * 128:(kt + 1) * 128],
                                start=(kt == 0), stop=(kt == KT - 1))
                        nc.vector.tensor_copy(
                            out=v4[b][:, tt, :, 0:64],
                            in_=vacc[:, 0:128].rearrange("p (h e) -> p h e", h=2))
                    for h in range(2):
                        nc.gpsimd.tensor_copy(
                            out=v4[b][:, tt0:tt1, h, 64:65],
                            in_=onesb_sb[:, 0:tt1 - tt0].rearrange(
                                "p (t o) -> p t o", o=1))

                def k_pass(b, w_sb, dst, tc0, tc1):
                    for tch in range(tc0, tc1):
                        kacc = pmm.tile([128, 512], F32, tag="mm",
                                        name=f"kacc_{id(dst)}_{b}_{tch}_{_rep}")
                        for kt in range(KT):
                            nc.tensor.matmul(
                                kacc[:],
                                w_sb[:, kt * 128:(kt + 1) * 128],
                                x4[b][:, tch * 4:tch * 4 + 4, kt, :],
                                start=(kt == 0), stop=(kt == KT - 1))
                        nc.vector.tensor_copy(
                            out=dst[:, tch * 512:(tch + 1) * 512], in_=kacc[:])

                if free_av:
                    nc.vector.tensor_copy(out=efix[:], in_=x_sb[0][:, 0:1024])

                deferred = []  # (b, qc, stg) pending normalization

                def flush_norm():
                    while deferred:
                        fb, fqc, stg = deferred.pop(0)
                        for h in range(2):
                            lo = 64 * h
                            recip = work.tile([128, 512], F32R, tag="recip")
                            with nc.allow_low_precision(reason="softmax recip"):
                                nc.vector.reciprocal(
                                    out=recip[64:65, :],
                                    in_=stg[64:65, h * 512:(h + 1) * 512])
                            pbt = pmm.tile([128, 512], F32, tag="mm",
                                           name=f"pbt_{fb}_{fqc}_{h}_{_rep}")
                            pb = pbt[0:64, 0:512]
                            nc.tensor.matmul(
                                pb, onesf_sb[64:65, 0:64], recip[64:65, :],
                                start=True, stop=True)
                            nc.vector.tensor_mul(
                                out=aot[fb][lo:lo + 64,
                                            fqc * 512:(fqc + 1) * 512],
                                in0=stg[0:64, h * 512:(h + 1) * 512], in1=pb)

                def _issue_av(pav, b, item, n_kt):
                    kt, q0, e = item
                    for h in range(2):
                        nc.tensor.matmul(
                            pav[:, h * 512 + q0:h * 512 + 512],
                            v4[b][:, kt, h, :],
                            e[:, h * 512 + q0:h * 512 + 512],
                            start=(kt == 0), stop=(kt == n_kt - 1))

                def attn_qc(b, qc):
                    flush_norm()
                    pav = pav_pool.tile([65, 1024], F32, tag="pav",
                                        name=f"pav_{b}_{qc}_{_rep}")
                    n_kt = 4 * qc + 4
                    pend = []  # (kt, q0, e)
                    for kt in range(n_kt):
                        d = kt - 4 * qc
                        q0 = 128 * d if d > 0 else 0
                        st = pst.tile([128, 1024], F32, tag=f"st{kt % 2}",
                                      name=f"st_{b}_{qc}_{kt}_{_rep}")
                        diag = d >= 0
                        for h in range(2):
                            lo = 64 * h
                            nc.tensor.matmul(
                                st[:, h * 512 + q0:h * 512 + 512],
                                ktp[b][lo:lo + 64, kt * 128:(kt + 1) * 128],
                                qtp[b][lo:lo + 64, qc * 512 + q0:(qc + 1) * 512],
                                start=True, stop=not diag)
                            if diag:
                                nc.tensor.matmul(
                                    st[:, h * 512 + q0:h * 512 + q0 + 128],
                                    tri_sb[:], id_sb[:],
                                    start=False, stop=True)
                        if pend:
                            _issue_av(pav, b, pend.pop(0), n_kt)
                        e = epool.tile([128, 1024], BF16, tag=f"e{kt % 2}",
                                       name=f"e_{b}_{qc}_{kt}_{_rep}")
                        st3 = st[:].rearrange("p (h m) -> p h m", h=2)
                        e3 = e[:].rearrange("p (h m) -> p h m", h=2)
                        if dummy_e:
                            nc.vector.tensor_copy(
                                out=e[:, 0:1], in_=st[:, 0:1])
                        else:
                            nc.scalar.activation(
                                e3[:, :, q0:512], st3[:, :, q0:512],
                                mybir.ActivationFunctionType.Exp,
                                scale=SCALE)
                        pend.append((kt, q0, efix if free_av else e))
                    while pend:
                        _issue_av(pav, b, pend.pop(0), n_kt)
                    stg = work.tile([65, 1024], F32, tag="stg",
                                    name=f"stg_{b}_{qc}_{_rep}")
                    for h_ in range(2):  # one copy per PSUM bank
                        nc.vector.tensor_copy(
                            out=stg[:, h_ * 512:(h_ + 1) * 512],
                            in_=pav[:, h_ * 512:(h_ + 1) * 512])
                    deferred.append((b, qc, stg))

                # ---- schedule: just-in-time chunked feeds ----
                def feed(b, j):
                    v_pass(b, 4 * j, 4 * j + 4)
                    k_pass(b, wk_sb, ktp[b], j, j + 1)
                    k_pass(b, wq_sb, qtp[b], j, j + 1)

                if do_attn:
                    feed(0, 0)
                    attn_qc(0, 0)
                    feed(0, 1)
                    feed(1, 0)
                    attn_qc(0, 1)
                    feed(0, 2)
                    feed(1, 1)
                    attn_qc(0, 2)
                    feed(0, 3)
                    feed(1, 2)
                    attn_qc(0, 3)
                    feed(1, 3)
                    for qc in range(QC):
                        attn_qc(1, qc)
                    flush_norm()
                else:
                    for b_ in range(B):
                        for j in range(4):
                            feed(b_, j)

                # ---- A2A + projection ----
                if do_proj:
                    with tc.tile_pool(name=f"dram{_rep}", bufs=1,
                                      space="DRAM") as dram:
                        cin = dram.tile([8, 128, 512], BF16, tag="cin")
                        cout = dram.tile([8, 128, 512], BF16, tag="cout")
                        for j in range(8):
                            nc.sync.dma_start(
                                out=cin[j, :, :],
                                in_=aot[j // 4][:, (j % 4) * 512:(j % 4 + 1) * 512])
                        nc.gpsimd.collective_compute(
                            "AllToAll", mybir.AluOpType.bypass,
                            replica_groups=[list(range(8))],
                            ins=[cin[:].opt()], outs=[cout[:].opt()])
                        for s in range(8):
                            nc.sync.dma_start(out=pin[s][:], in_=cout[s, :, :])

                    for qi in range(4):
                        for fc in range(2):
                            ypt = pmm.tile([128, 512], F32, tag="mm",
                                           name=f"yp_{qi}_{fc}_{_rep}")
                            yp = ypt[:, 0:512]
                            for s in range(8):
                                nc.tensor.matmul(
                                    yp,
                                    pin[s][:, qi * 128:(qi + 1) * 128],
                                    wo_sb[:, s * DIM + fc * 512:s * DIM + fc * 512 + 512],
                                    start=(s == 0), stop=(s == 7))
                            y_sb = work.tile([128, 512], F32, tag="y")
                            nc.vector.tensor_copy(out=y_sb[:], in_=yp)
                            nc.sync.dma_start(
                                out=out[qi * 128:(qi + 1) * 128,
                                        fc * 512:fc * 512 + 512],
                                in_=y_sb[:])
                elif do_attn:
                    for b_ in range(B):
                        y_sb = work.tile([128, 512], F32, tag="y")
                        nc.vector.tensor_copy(out=y_sb[:], in_=aot[b_][:, 0:512])
                        nc.sync.dma_start(out=out[0:128, 0:512], in_=y_sb[:])
                else:
                    for b_ in range(B):
                        y_sb = work.tile([128, 512], F32, tag="y")
                        nc.vector.tensor_copy(out=y_sb[:, 0:128], in_=v_sb[b_][:, 0:128])
                        nc.vector.tensor_copy(out=y_sb[:, 128:256], in_=ktp[b_][:, 0:128])
                        nc.vector.tensor_copy(out=y_sb[:, 256:384], in_=qtp[b_][:, 0:128])
                        nc.sync.dma_start(out=out[0:128, 0:512], in_=y_sb[:])

    nc.finalize()
    return nc


def make_in_maps(x, W_qkv, W_proj):
    import ml_dtypes

    bf = ml_dtypes.bfloat16
    x = np.asarray(x, dtype=np.float32)
    W_qkv = np.asarray(W_qkv, dtype=np.float32)
    W_proj = np.asarray(W_proj, dtype=np.float32)
    W_q, W_k, W_v = W_qkv[:DIM], W_qkv[DIM:2 * DIM], W_qkv[2 * DIM:]

    def pair_major(W):
        # [128 cin-part, pair, kt, 128 dims]; value = W[pair*128+d, kt*128+p]
        WT = W.T.reshape(KT, 128, 8, 128)
        return np.ascontiguousarray(WT.transpose(1, 2, 0, 3))

    wq_pm = pair_major(W_q).astype(bf)
    wk_pm = pair_major(W_k).astype(bf)
    wv_pm = pair_major(W_v).astype(bf)
    wo_d = np.ascontiguousarray(
        W_proj.T.reshape(KT, 128, DIM).transpose(1, 0, 2).reshape(128, KT * DIM)
    ).astype(bf)

    xTl = []
    for b in range(B):
        xb = x[b]
        xTl.append(np.ascontiguousarray(
            xb.T.reshape(KT, 128, TT, 128).transpose(1, 2, 0, 3)
            .reshape(128, KT * T)).astype(bf))

    idx = np.arange(128)
    tri_d = (-30000.0 * (idx[None, :] > idx[:, None])).astype(bf)
    allm_d = np.full((128, 128), -30000.0, np.float32).astype(bf)
    id_d = np.eye(128, dtype=np.float32).astype(bf)

    in_maps = []
    for core in range(N_CORES):
        in_maps.append({
            "xT0": xTl[0], "xT1": xTl[1],
            "wq": np.ascontiguousarray(wq_pm[:, core]).reshape(128, KT * 128),
            "wk": np.ascontiguousarray(wk_pm[:, core]).reshape(128, KT * 128),
            "wv": np.ascontiguousarray(wv_pm[:, core]).reshape(128, KT * 128),
            "wo": wo_d,
            "tri": tri_d, "allm": allm_d, "ident": id_d,
            "ones_bf": np.ones((128, 64), bf),
            "ones_fr": np.ones((128, 64), np.float32),
        })
    return in_maps


def assemble_output(results):
    y = np.empty((B, T, DIM), dtype=np.float32)
    for core in range(N_CORES):
        b, r = core // 4, core % 4
        y[b, 512 * r:512 * (r + 1)] = results[core]["out"]
    return y


def kernel(x, W_qkv, W_proj):
    from concourse.bass_utils import run_bass_kernel_spmd

    if "nc" not in _CACHE:
        _CACHE["nc"] = build_nc()
    nc = _CACHE["nc"]
    in_maps = make_in_maps(x, W_qkv, W_proj)
    res = run_bass_kernel_spmd(nc, in_maps, list(range(N_CORES)))
    return assemble_output(res.results)
